# revision 6
# baseline (speedup 1.0000x reference)
"""Ensemble-MLP (grouped 1x1 conv) Trainium2 kernel.

Computation (per batch row b):
  h = relu(x @ W0[e] + b0[e])             e = 0..9 ensembles, 256 units
  h = relu(h @ Wh[l,e] + bh[l,e])         l = 0..6 hidden layers
  y[e] = h @ Wf[e] + bf[e]                201 outputs per ensemble
  out[b, o'] = mean_j yflat[b, o'*10 + j] (strided channel mix, yflat = e*201+o)

Strategy (v2):
  * Data parallel: batch 16384 -> 2048 rows per core on 8 cores. Weights
    replicated, no collectives.
  * bf16 operands everywhere on the PE (same 1 col/cycle rate as fp32r, but
    FWL weight loads, half the DMA/SBUF traffic); fp32 PSUM accumulation.
  * Activations live transposed in SBUF: H[channel, batch], 2 chunks of 128
    channels x 2048 batch. Every layer: matmul(psum[o,b] += W[c,o].T @ H[c,b]).
  * PSUM tiles are [128, 1024] (2 banks); matmuls write 512-col bank slices,
    ONE relu per 2-bank tile (halves ACT/DVE instruction count). bt-major
    matmul order so each layer's relus complete before the next layer needs
    them (no PE stalls). oc0 relus on ScalarE, oc1 relus on VectorE.
  * Layer-0 bias folded into the matmul via an all-ones row on x^T (K=7).
    Hidden biases ride the relu post-op.
  * Final channel-mixing mean folded into the last-layer weights on the host
    (exact linear algebra). Final layer runs as a separate phase at the end:
    layer-7 activations for all 10 ensembles are kept in SBUF (bf16, 10MB)
    and the ensemble sum accumulates IN PSUM (start/stop over 20 matmuls per
    bank slice) -- no vector-engine accumulation ops at all.
  * PE pre-warm: a few dummy matmuls on a memset scratch tile right after
    the framework preamble so the HAM clock-gate opens before real work.
"""

import numpy as np
from contextlib import ExitStack

import ml_dtypes
import concourse.bass as bass
import concourse.mybir as mybir
import concourse.tile as tile
from concourse import bacc, bass_utils

F32 = mybir.dt.float32
BF16 = mybir.dt.bfloat16
BF16_NP = ml_dtypes.bfloat16

ENS, N_UNITS, N_HID, IN_DIM, OUT_DIM, BATCH = 10, 256, 7, 6, 201, 16384
N_CORES = 8
BC = BATCH // N_CORES          # 2048 batch rows per core
N_WARM = 8                     # pre-warm matmuls for the HAM clock gate

_CACHE = {}


def build_program():
    nc = bacc.Bacc("TRN2", debug=False)

    xt = nc.dram_tensor("xt", (IN_DIM + 1, BC), BF16, kind="ExternalInput").ap()
    w0 = nc.dram_tensor("w0", (ENS, IN_DIM + 1, N_UNITS), BF16, kind="ExternalInput").ap()
    wh = nc.dram_tensor("wh", (ENS, 128, N_HID * 2 * N_UNITS), BF16, kind="ExternalInput").ap()
    bh = nc.dram_tensor("bh", (ENS, 128, N_HID * 2), F32, kind="ExternalInput").ap()
    vw = nc.dram_tensor("vw", (ENS, 128, 2 * 256), BF16, kind="ExternalInput").ap()
    bp = nc.dram_tensor("bp", (128, 2), F32, kind="ExternalInput").ap()
    yt = nc.dram_tensor("yt", (256, BC), F32, kind="ExternalOutput").ap()

    add = mybir.AluOpType.add
    mx = mybir.AluOpType.max
    relu = mybir.ActivationFunctionType.Relu
    ident = mybir.ActivationFunctionType.Identity

    with ExitStack() as ctx:
        tc = ctx.enter_context(tile.TileContext(nc))
        const = ctx.enter_context(tc.tile_pool(name="const", bufs=1))
        wpool = ctx.enter_context(tc.tile_pool(name="w", bufs=2))
        vpool = ctx.enter_context(tc.tile_pool(name="v", bufs=1))
        hpool = ctx.enter_context(tc.tile_pool(name="h", bufs=2))
        hfpool = ctx.enter_context(tc.tile_pool(name="hf", bufs=1))
        spool = ctx.enter_context(tc.tile_pool(name="stage", bufs=1))
        pspool = ctx.enter_context(tc.tile_pool(name="ps", bufs=4, space="PSUM"))

        x_t = const.tile([IN_DIM + 1, BC], BF16)
        bp_t = const.tile([128, 2], F32)
        scratch = const.tile([128, 512], BF16)
        v_all = vpool.tile([128, ENS * 512], BF16)
        hf = {}

        # ---- PE pre-warm: dummy matmuls on zeroed scratch, result unread ----
        nc.gpsimd.memset(scratch, 0.0)
        ps_warm = pspool.tile([128, 1024], F32, tag="ps")
        for _ in range(N_WARM):
            nc.tensor.matmul(ps_warm[:, 0:512], lhsT=scratch[:, 0:128],
                             rhs=scratch, start=True, stop=True)

        def relu_tile(engine_is_act, dst, ps, bias_ap):
            if engine_is_act:
                nc.scalar.activation(out=dst, in_=ps, func=relu,
                                     bias=bias_ap if bias_ap is not None else 0.0)
            elif bias_ap is not None:
                nc.vector.tensor_scalar(out=dst, in0=ps, scalar1=bias_ap,
                                        scalar2=0.0, op0=add, op1=mx)
            else:
                nc.vector.tensor_scalar(out=dst, in0=ps, scalar1=0.0,
                                        scalar2=None, op0=mx)

        for e in range(ENS):
            w0_t = wpool.tile([IN_DIM + 1, N_UNITS], BF16, tag="w0")
            nc.sync.dma_start(out=w0_t, in_=w0[e])
            if e == 0:
                # x lands on a single DMA engine (only 7 partition rows):
                # issue in b-slices so layer-0's first matmuls wait only on
                # their own slice.
                for bt in range(4):
                    nc.sync.dma_start(out=x_t[:, bt * 512:(bt + 1) * 512],
                                      in_=xt[:, bt * 512:(bt + 1) * 512])
                nc.sync.dma_start(out=bp_t, in_=bp)
            wh_t = wpool.tile([128, N_HID * 2 * N_UNITS], BF16, tag="wh")
            for l in range(N_HID):
                nc.gpsimd.dma_start(out=wh_t[:, l * 512:(l + 1) * 512],
                                    in_=wh[e][:, l * 512:(l + 1) * 512])
            bh_t = wpool.tile([128, N_HID * 2], F32, tag="bh")
            nc.sync.dma_start(out=bh_t, in_=bh[e])
            nc.gpsimd.dma_start(out=v_all[:, e * 512:(e + 1) * 512], in_=vw[e])

            # ---- layer 0: x^T (7, BC) -> h (2x128, BC); bias folded in ----
            h_cur = [hpool.tile([128, BC], BF16, tag=f"h{kc}", name=f"h{kc}_e{e}")
                     for kc in range(2)]
            ps_t = {}
            for j in range(2):           # bt half: columns j*1024..+1024
                for oc in range(2):
                    ps_t[(oc, j)] = pspool.tile([128, 1024], F32, tag="ps",
                                                name=f"ps{oc}_{j}_e{e}L0")
            for bt in range(4):
                j, s = bt // 2, bt % 2
                sl = slice(s * 512, (s + 1) * 512)
                xsl = slice(bt * 512, (bt + 1) * 512)
                for oc in range(2):
                    nc.tensor.matmul(ps_t[(oc, j)][:, sl],
                                     lhsT=w0_t[:, oc * 128:(oc + 1) * 128],
                                     rhs=x_t[:, xsl], start=True, stop=True)
                if s == 1:
                    for oc in range(2):
                        relu_tile(oc == 0, h_cur[oc][:, j * 1024:(j + 1) * 1024],
                                  ps_t[(oc, j)], None)

            # ---- 7 hidden layers: K=256 (2 chunks), M=256 (2 chunks) ----
            for l in range(N_HID):
                if l < N_HID - 1:
                    h_nxt = [hpool.tile([128, BC], BF16, tag=f"h{kc}",
                                        name=f"h{kc}_e{e}l{l}")
                             for kc in range(2)]
                else:
                    h_nxt = [hfpool.tile([128, BC], BF16, tag=f"hf{e}_{kc}",
                                         name=f"hf{e}_{kc}")
                             for kc in range(2)]
                    for kc in range(2):
                        hf[(e, kc)] = h_nxt[kc]
                base = l * 2 * N_UNITS
                ps_t = {}
                for j in range(2):
                    for oc in range(2):
                        ps_t[(oc, j)] = pspool.tile([128, 1024], F32, tag="ps",
                                                    name=f"ps{oc}_{j}_e{e}l{l}")
                for bt in range(4):
                    j, s = bt // 2, bt % 2
                    sl = slice(s * 512, (s + 1) * 512)
                    hsl = slice(bt * 512, (bt + 1) * 512)
                    for oc in range(2):
                        c0 = base + oc * 128
                        c1 = base + N_UNITS + oc * 128
                        nc.tensor.matmul(ps_t[(oc, j)][:, sl],
                                         lhsT=wh_t[:, c0:c0 + 128],
                                         rhs=h_cur[0][:, hsl],
                                         start=True, stop=False)
                        nc.tensor.matmul(ps_t[(oc, j)][:, sl],
                                         lhsT=wh_t[:, c1:c1 + 128],
                                         rhs=h_cur[1][:, hsl],
                                         start=False, stop=True)
                    if s == 1:
                        for oc in range(2):
                            relu_tile(oc == 0,
                                      h_nxt[oc][:, j * 1024:(j + 1) * 1024],
                                      ps_t[(oc, j)],
                                      bh_t[:, l * 2 + oc:l * 2 + oc + 1])
                h_cur = h_nxt

        # ---- final layer: out[o', b] = sum_e sum_kc V[e][kc].T @ hf[e][kc] ----
        # Ensemble sum accumulates in PSUM (20 matmuls per bank slice).
        for g, (j, oc) in enumerate([(0, 0), (0, 1), (1, 0), (1, 1)]):
            ps = pspool.tile([128, 1024], F32, tag="ps", name=f"psf{g}")
            for s in range(2):
                bt = 2 * j + s
                hsl = slice(bt * 512, (bt + 1) * 512)
                sl = slice(s * 512, (s + 1) * 512)
                for e in range(ENS):
                    for kc in range(2):
                        c = e * 512 + kc * 256 + oc * 128
                        nc.tensor.matmul(ps[:, sl],
                                         lhsT=v_all[:, c:c + 128],
                                         rhs=hf[(e, kc)][:, hsl],
                                         start=(e == 0 and kc == 0),
                                         stop=(e == ENS - 1 and kc == 1))
            stage = spool.tile([128, 1024], F32, tag=f"s{g}")
            if g % 2 == 0:
                nc.scalar.activation(out=stage, in_=ps, func=ident,
                                     bias=bp_t[:, oc:oc + 1])
            else:
                nc.vector.tensor_scalar(out=stage, in0=ps,
                                        scalar1=bp_t[:, oc:oc + 1],
                                        scalar2=None, op0=add)
            dma_eng = [nc.sync, nc.gpsimd, nc.scalar, nc.sync][g]
            dma_eng.dma_start(out=yt[oc * 128:(oc + 1) * 128,
                                     j * 1024:(j + 1) * 1024], in_=stage)

    nc.compile()
    return nc


def prepare_inputs(x, W0, b0, Wh, bh, Wf, bf):
    """Host-side weight refactoring + per-core sharding. Exact fp32 linear
    algebra for the folds; bf16 quantization only at the very end."""
    x = np.asarray(x, np.float32)
    W0 = np.asarray(W0, np.float32)
    b0 = np.asarray(b0, np.float32)
    Wh = np.asarray(Wh, np.float32)
    bh = np.asarray(bh, np.float32)
    Wf = np.asarray(Wf, np.float32)
    bf = np.asarray(bf, np.float32)

    # layer 0 with bias folded: lhsT rows = 6 inputs + ones row
    w0a = np.concatenate([W0, b0[:, None, :]], axis=1)  # (ENS, 7, 256)
    w0a = np.ascontiguousarray(w0a).astype(BF16_NP)

    # hidden weights -> [e, p, (l, kc, o)]
    whh = (Wh.transpose(1, 0, 2, 3)              # (e, l, h, o)
             .reshape(ENS, N_HID, 2, 128, N_UNITS)
             .transpose(0, 3, 1, 2, 4)           # (e, p, l, kc, o)
             .reshape(ENS, 128, N_HID * 2 * N_UNITS))
    whh = np.ascontiguousarray(whh).astype(BF16_NP)

    # hidden biases -> [e, p, (l, oc)]
    bhh = (bh.transpose(1, 0, 2)                 # (e, l, o)
             .reshape(ENS, N_HID, 2, 128)
             .transpose(0, 3, 1, 2)              # (e, p, l, oc)
             .reshape(ENS, 128, N_HID * 2))
    bhh = np.ascontiguousarray(bhh)

    # fold the strided channel-mix mean into the final weights:
    # out[b, o'] = 0.1 * sum_j yflat[b, o'*10+j],  yflat col c = e*201+o
    C = ENS * OUT_DIM
    M = np.zeros((C, OUT_DIM), np.float32)
    M[np.arange(C), np.arange(C) // ENS] = 1.0 / ENS
    Me = M.reshape(ENS, OUT_DIM, OUT_DIM)
    V = np.einsum('eho,eoc->ehc', Wf, Me)        # (ENS, 256, 201)
    bpv = bf.reshape(C) @ M                      # (201,)

    Vp = np.zeros((ENS, N_UNITS, 256), np.float32)
    Vp[:, :, :OUT_DIM] = V
    vww = (Vp.reshape(ENS, 2, 128, 256)
             .transpose(0, 2, 1, 3)              # (e, p, kc, o')
             .reshape(ENS, 128, 2 * 256))
    vww = np.ascontiguousarray(vww).astype(BF16_NP)

    bp_pad = np.zeros(256, np.float32)
    bp_pad[:OUT_DIM] = bpv
    bp_t = np.ascontiguousarray(bp_pad.reshape(2, 128).T)  # (128, 2)

    ones = np.ones((1, BC), np.float32)
    in_maps = []
    for c in range(N_CORES):
        xs = x[c * BC:(c + 1) * BC]              # (BC, 6)
        xt = np.ascontiguousarray(
            np.concatenate([xs.T, ones], axis=0)).astype(BF16_NP)  # (7, BC)
        in_maps.append({
            "xt": xt, "w0": w0a, "wh": whh, "bh": bhh, "vw": vww, "bp": bp_t,
        })
    return in_maps


def run(in_maps, trace=False, tmpdir=None):
    if "nc" not in _CACHE:
        _CACHE["nc"] = build_program()
    nc = _CACHE["nc"]
    res = bass_utils.run_bass_kernel_spmd(
        nc, in_maps, core_ids=list(range(N_CORES)), trace=trace, tmpdir=tmpdir)
    return res


def kernel(x, W0, b0, Wh, bh, Wf, bf):
    in_maps = prepare_inputs(x, W0, b0, Wh, bh, Wf, bf)
    res = run(in_maps)
    out = np.empty((BATCH, OUT_DIM), np.float32)
    for c in range(N_CORES):
        out[c * BC:(c + 1) * BC, :] = res.results[c]["yt"][:OUT_DIM].T
    return out


# revision 12
# speedup vs baseline: 1.2001x; 1.2001x over previous
"""Ensemble-MLP (grouped 1x1 conv) Trainium2 kernel.

Computation (per batch row b):
  h = relu(x @ W0[e] + b0[e])             e = 0..9 ensembles, 256 units
  h = relu(h @ Wh[l,e] + bh[l,e])         l = 0..6 hidden layers
  y[e] = h @ Wf[e] + bf[e]                201 outputs per ensemble
  out[b, o'] = mean_j yflat[b, o'*10 + j] (strided channel mix, yflat = e*201+o)

Strategy (v2):
  * Data parallel: batch 16384 -> 2048 rows per core on 8 cores. Weights
    replicated, no collectives.
  * bf16 operands everywhere on the PE (same 1 col/cycle rate as fp32r, but
    FWL weight loads, half the DMA/SBUF traffic); fp32 PSUM accumulation.
  * Activations live transposed in SBUF: H[channel, batch], 2 chunks of 128
    channels x 2048 batch. Every layer: matmul(psum[o,b] += W[c,o].T @ H[c,b]).
  * PSUM tiles are [128, 1024] (2 banks); matmuls write 512-col bank slices,
    ONE relu per 2-bank tile (halves ACT/DVE instruction count). bt-major
    matmul order so each layer's relus complete before the next layer needs
    them (no PE stalls). oc0 relus on ScalarE, oc1 relus on VectorE.
  * Layer-0 bias folded into the matmul via an all-ones row on x^T (K=7).
    Hidden biases ride the relu post-op.
  * Final channel-mixing mean folded into the last-layer weights on the host
    (exact linear algebra). Final layer runs as a separate phase at the end:
    layer-7 activations for all 10 ensembles are kept in SBUF (bf16, 10MB)
    and the ensemble sum accumulates IN PSUM (start/stop over 20 matmuls per
    bank slice) -- no vector-engine accumulation ops at all.
  * PE pre-warm: a few dummy matmuls on a memset scratch tile right after
    the framework preamble so the HAM clock-gate opens before real work.
"""

import numpy as np
from contextlib import ExitStack

import ml_dtypes
import concourse.bass as bass
import concourse.mybir as mybir
import concourse.tile as tile
from concourse import bacc, bass_utils

F32 = mybir.dt.float32
BF16 = mybir.dt.bfloat16
BF16_NP = ml_dtypes.bfloat16

ENS, N_UNITS, N_HID, IN_DIM, OUT_DIM, BATCH = 10, 256, 7, 6, 201, 16384
N_CORES = 8
BC = BATCH // N_CORES          # 2048 batch rows per core
N_WARM = 8                     # pre-warm matmuls for the HAM clock gate

_CACHE = {}


def build_program():
    nc = bacc.Bacc("TRN2", debug=False)

    # x^T (+ones row) replicated at partition offsets 0/32/64/96 so layer-0's
    # K=7 matmuls can run 4-concurrent via PE row-group tiling.
    xq = nc.dram_tensor("xq", (128, BC), BF16, kind="ExternalInput").ap()
    w0 = nc.dram_tensor("w0", (ENS, 128, 128), BF16, kind="ExternalInput").ap()
    wh = nc.dram_tensor("wh", (ENS, 128, N_HID * 2 * N_UNITS), BF16, kind="ExternalInput").ap()
    bh = nc.dram_tensor("bh", (ENS, 128, N_HID * 2), F32, kind="ExternalInput").ap()
    vw = nc.dram_tensor("vw", (ENS, 128, 2 * 256), BF16, kind="ExternalInput").ap()
    bp = nc.dram_tensor("bp", (128, 2), F32, kind="ExternalInput").ap()
    yt = nc.dram_tensor("yt", (256, BC), F32, kind="ExternalOutput").ap()

    add = mybir.AluOpType.add
    mx = mybir.AluOpType.max
    relu = mybir.ActivationFunctionType.Relu
    ident = mybir.ActivationFunctionType.Identity

    with ExitStack() as ctx:
        tc = ctx.enter_context(tile.TileContext(nc))
        const = ctx.enter_context(tc.tile_pool(name="const", bufs=1))
        wpool = ctx.enter_context(tc.tile_pool(name="w", bufs=2))
        vpool = ctx.enter_context(tc.tile_pool(name="v", bufs=1))
        hpool = ctx.enter_context(tc.tile_pool(name="h", bufs=2))
        hfpool = ctx.enter_context(tc.tile_pool(name="hf", bufs=1))
        spool = ctx.enter_context(tc.tile_pool(name="stage", bufs=1))
        pspool = ctx.enter_context(tc.tile_pool(name="ps", bufs=4, space="PSUM"))

        x_t = const.tile([128, BC], BF16)
        bp_t = const.tile([128, 2], F32)
        scratch = const.tile([128, 512], BF16)
        v_all = vpool.tile([128, ENS * 512], BF16)
        hf = {}

        # ---- PE pre-warm: dummy matmuls on zeroed scratch, result unread ----
        nc.gpsimd.memset(scratch, 0.0)
        ps_warm = pspool.tile([128, 1024], F32, tag="ps")
        for _ in range(N_WARM):
            nc.tensor.matmul(ps_warm[:, 0:512], lhsT=scratch[:, 0:128],
                             rhs=scratch, start=True, stop=True)

        def relu_tile(engine_is_act, dst, ps, bias_ap):
            if engine_is_act:
                nc.scalar.activation(out=dst, in_=ps, func=relu,
                                     bias=bias_ap if bias_ap is not None else 0.0)
            elif bias_ap is not None:
                nc.vector.tensor_scalar(out=dst, in0=ps, scalar1=bias_ap,
                                        scalar2=0.0, op0=add, op1=mx)
            else:
                nc.vector.tensor_scalar(out=dst, in0=ps, scalar1=0.0,
                                        scalar2=None, op0=mx)

        for e in range(ENS):
            w0_t = wpool.tile([128, 128], BF16, tag="w0")
            nc.sync.dma_start(out=w0_t, in_=w0[e])
            if e == 0:
                # issue x in b-slices so layer-0's first matmuls wait only on
                # their own slice.
                for bt in range(4):
                    nc.sync.dma_start(out=x_t[:, bt * 512:(bt + 1) * 512],
                                      in_=xq[:, bt * 512:(bt + 1) * 512])
                nc.sync.dma_start(out=bp_t, in_=bp)
            wh_t = wpool.tile([128, N_HID * 2 * N_UNITS], BF16, tag="wh")
            for l in range(N_HID):
                nc.gpsimd.dma_start(out=wh_t[:, l * 512:(l + 1) * 512],
                                    in_=wh[e][:, l * 512:(l + 1) * 512])
            bh_t = wpool.tile([128, N_HID * 2], F32, tag="bh")
            nc.sync.dma_start(out=bh_t, in_=bh[e])
            nc.gpsimd.dma_start(out=v_all[:, e * 512:(e + 1) * 512], in_=vw[e])

            # ---- layer 0: x^T (7, BC) -> h (2x128, BC); bias folded in.
            # 4 K=7 matmuls run concurrently in the 4 PE row groups
            # (row-group i holds weights for oc=i//2, streams bt parity i%2).
            h_cur = [hpool.tile([128, BC], BF16, tag=f"h{kc}", name=f"h{kc}_e{e}")
                     for kc in range(2)]
            ps_t = {}
            for j in range(2):           # bt half: columns j*1024..+1024
                for oc in range(2):
                    ps_t[(oc, j)] = pspool.tile([128, 1024], F32, tag="ps",
                                                name=f"ps{oc}_{j}_e{e}L0")
            for j in range(2):
                for i in range(4):
                    oc, p = i // 2, i % 2
                    bt = 2 * j + p
                    nc.tensor.matmul(
                        ps_t[(oc, j)][:, p * 512:(p + 1) * 512],
                        lhsT=w0_t[32 * i:32 * i + IN_DIM + 1, :],
                        rhs=x_t[32 * i:32 * i + IN_DIM + 1,
                                bt * 512:(bt + 1) * 512],
                        start=True, stop=True, tile_position=(32 * i, 0))
                for oc in range(2):
                    relu_tile(oc == 0, h_cur[oc][:, j * 1024:(j + 1) * 1024],
                              ps_t[(oc, j)], None)

            # ---- 7 hidden layers: K=256 (2 chunks), M=256 (2 chunks) ----
            for l in range(N_HID):
                if l < N_HID - 1:
                    h_nxt = [hpool.tile([128, BC], BF16, tag=f"h{kc}",
                                        name=f"h{kc}_e{e}l{l}")
                             for kc in range(2)]
                else:
                    h_nxt = [hfpool.tile([128, BC], BF16, tag=f"hf{e}_{kc}",
                                         name=f"hf{e}_{kc}")
                             for kc in range(2)]
                    for kc in range(2):
                        hf[(e, kc)] = h_nxt[kc]
                base = l * 2 * N_UNITS
                ps_t = {}
                for j in range(2):
                    for oc in range(2):
                        ps_t[(oc, j)] = pspool.tile([128, 1024], F32, tag="ps",
                                                    name=f"ps{oc}_{j}_e{e}l{l}")
                for bt in range(4):
                    j, s = bt // 2, bt % 2
                    sl = slice(s * 512, (s + 1) * 512)
                    hsl = slice(bt * 512, (bt + 1) * 512)
                    for oc in range(2):
                        c0 = base + oc * 128
                        c1 = base + N_UNITS + oc * 128
                        nc.tensor.matmul(ps_t[(oc, j)][:, sl],
                                         lhsT=wh_t[:, c0:c0 + 128],
                                         rhs=h_cur[0][:, hsl],
                                         start=True, stop=False)
                        nc.tensor.matmul(ps_t[(oc, j)][:, sl],
                                         lhsT=wh_t[:, c1:c1 + 128],
                                         rhs=h_cur[1][:, hsl],
                                         start=False, stop=True)
                    if s == 1:
                        for oc in range(2):
                            relu_tile(oc == 0,
                                      h_nxt[oc][:, j * 1024:(j + 1) * 1024],
                                      ps_t[(oc, j)],
                                      bh_t[:, l * 2 + oc:l * 2 + oc + 1])
                h_cur = h_nxt

        # ---- final layer: out[o', b] = sum_e sum_kc V[e][kc].T @ hf[e][kc] ----
        # Ensemble sum accumulates in PSUM (20 matmuls per bank slice).
        for g, (j, oc) in enumerate([(0, 0), (0, 1), (1, 0), (1, 1)]):
            ps = pspool.tile([128, 1024], F32, tag="ps", name=f"psf{g}")
            for s in range(2):
                bt = 2 * j + s
                hsl = slice(bt * 512, (bt + 1) * 512)
                sl = slice(s * 512, (s + 1) * 512)
                for e in range(ENS):
                    for kc in range(2):
                        c = e * 512 + kc * 256 + oc * 128
                        nc.tensor.matmul(ps[:, sl],
                                         lhsT=v_all[:, c:c + 128],
                                         rhs=hf[(e, kc)][:, hsl],
                                         start=(e == 0 and kc == 0),
                                         stop=(e == ENS - 1 and kc == 1))
            stage = spool.tile([128, 1024], F32, tag=f"s{g}")
            if g % 2 == 0:
                nc.scalar.activation(out=stage, in_=ps, func=ident,
                                     bias=bp_t[:, oc:oc + 1])
            else:
                nc.vector.tensor_scalar(out=stage, in0=ps,
                                        scalar1=bp_t[:, oc:oc + 1],
                                        scalar2=None, op0=add)
            dma_eng = [nc.sync, nc.gpsimd, nc.scalar, nc.sync][g]
            dma_eng.dma_start(out=yt[oc * 128:(oc + 1) * 128,
                                     j * 1024:(j + 1) * 1024], in_=stage)

    nc.compile()
    return nc


def prepare_inputs(x, W0, b0, Wh, bh, Wf, bf):
    """Host-side weight refactoring + per-core sharding. Exact fp32 linear
    algebra for the folds; bf16 quantization only at the very end."""
    x = np.asarray(x, np.float32)
    W0 = np.asarray(W0, np.float32)
    b0 = np.asarray(b0, np.float32)
    Wh = np.asarray(Wh, np.float32)
    bh = np.asarray(bh, np.float32)
    Wf = np.asarray(Wf, np.float32)
    bf = np.asarray(bf, np.float32)

    # layer 0 with bias folded: lhsT rows = 6 inputs + ones row; packed into
    # the 4 PE row groups (groups 0,1 -> oc0 weights; groups 2,3 -> oc1).
    w0a = np.concatenate([W0, b0[:, None, :]], axis=1)  # (ENS, 7, 256)
    w0q = np.zeros((ENS, 128, 128), np.float32)
    for i in range(4):
        w0q[:, 32 * i:32 * i + IN_DIM + 1, :] = \
            w0a[:, :, (i // 2) * 128:(i // 2) * 128 + 128]
    w0q = np.ascontiguousarray(w0q).astype(BF16_NP)

    # hidden weights -> [e, p, (l, kc, o)]
    whh = (Wh.transpose(1, 0, 2, 3)              # (e, l, h, o)
             .reshape(ENS, N_HID, 2, 128, N_UNITS)
             .transpose(0, 3, 1, 2, 4)           # (e, p, l, kc, o)
             .reshape(ENS, 128, N_HID * 2 * N_UNITS))
    whh = np.ascontiguousarray(whh).astype(BF16_NP)

    # hidden biases -> [e, p, (l, oc)]
    bhh = (bh.transpose(1, 0, 2)                 # (e, l, o)
             .reshape(ENS, N_HID, 2, 128)
             .transpose(0, 3, 1, 2)              # (e, p, l, oc)
             .reshape(ENS, 128, N_HID * 2))
    bhh = np.ascontiguousarray(bhh)

    # fold the strided channel-mix mean into the final weights:
    # out[b, o'] = 0.1 * sum_j yflat[b, o'*10+j],  yflat col c = e*201+o
    C = ENS * OUT_DIM
    M = np.zeros((C, OUT_DIM), np.float32)
    M[np.arange(C), np.arange(C) // ENS] = 1.0 / ENS
    Me = M.reshape(ENS, OUT_DIM, OUT_DIM)
    V = np.einsum('eho,eoc->ehc', Wf, Me)        # (ENS, 256, 201)
    bpv = bf.reshape(C) @ M                      # (201,)

    Vp = np.zeros((ENS, N_UNITS, 256), np.float32)
    Vp[:, :, :OUT_DIM] = V
    vww = (Vp.reshape(ENS, 2, 128, 256)
             .transpose(0, 2, 1, 3)              # (e, p, kc, o')
             .reshape(ENS, 128, 2 * 256))
    vww = np.ascontiguousarray(vww).astype(BF16_NP)

    bp_pad = np.zeros(256, np.float32)
    bp_pad[:OUT_DIM] = bpv
    bp_t = np.ascontiguousarray(bp_pad.reshape(2, 128).T)  # (128, 2)

    ones = np.ones((1, BC), np.float32)
    in_maps = []
    for c in range(N_CORES):
        xs = x[c * BC:(c + 1) * BC]              # (BC, 6)
        xt = np.concatenate([xs.T, ones], axis=0)  # (7, BC)
        xqc = np.zeros((128, BC), np.float32)
        for i in range(4):
            xqc[32 * i:32 * i + IN_DIM + 1, :] = xt
        xqc = np.ascontiguousarray(xqc).astype(BF16_NP)
        in_maps.append({
            "xq": xqc, "w0": w0q, "wh": whh, "bh": bhh, "vw": vww, "bp": bp_t,
        })
    return in_maps


def run(in_maps, trace=False, tmpdir=None):
    if "nc" not in _CACHE:
        _CACHE["nc"] = build_program()
    nc = _CACHE["nc"]
    res = bass_utils.run_bass_kernel_spmd(
        nc, in_maps, core_ids=list(range(N_CORES)), trace=trace, tmpdir=tmpdir)
    return res


def kernel(x, W0, b0, Wh, bh, Wf, bf):
    in_maps = prepare_inputs(x, W0, b0, Wh, bh, Wf, bf)
    res = run(in_maps)
    out = np.empty((BATCH, OUT_DIM), np.float32)
    for c in range(N_CORES):
        out[c * BC:(c + 1) * BC, :] = res.results[c]["yt"][:OUT_DIM].T
    return out


# revision 16
# speedup vs baseline: 1.2229x; 1.0190x over previous
"""Ensemble-MLP (grouped 1x1 conv) Trainium2 kernel.

Computation (per batch row b):
  h = relu(x @ W0[e] + b0[e])             e = 0..9 ensembles, 256 units
  h = relu(h @ Wh[l,e] + bh[l,e])         l = 0..6 hidden layers
  y[e] = h @ Wf[e] + bf[e]                201 outputs per ensemble
  out[b, o'] = mean_j yflat[b, o'*10 + j] (strided channel mix, yflat = e*201+o)

Strategy (v2):
  * Data parallel: batch 16384 -> 2048 rows per core on 8 cores. Weights
    replicated, no collectives.
  * bf16 operands everywhere on the PE (same 1 col/cycle rate as fp32r, but
    FWL weight loads, half the DMA/SBUF traffic); fp32 PSUM accumulation.
  * Activations live transposed in SBUF: H[channel, batch], 2 chunks of 128
    channels x 2048 batch. Every layer: matmul(psum[o,b] += W[c,o].T @ H[c,b]).
  * PSUM tiles are [128, 1024] (2 banks); matmuls write 512-col bank slices,
    ONE relu per 2-bank tile (halves ACT/DVE instruction count). bt-major
    matmul order so each layer's relus complete before the next layer needs
    them (no PE stalls). oc0 relus on ScalarE, oc1 relus on VectorE.
  * Layer-0 bias folded into the matmul via an all-ones row on x^T (K=7).
    Hidden biases ride the relu post-op.
  * Final channel-mixing mean folded into the last-layer weights on the host
    (exact linear algebra). Final layer runs as a separate phase at the end:
    layer-7 activations for all 10 ensembles are kept in SBUF (bf16, 10MB)
    and the ensemble sum accumulates IN PSUM (start/stop over 20 matmuls per
    bank slice) -- no vector-engine accumulation ops at all.
  * PE pre-warm: a few dummy matmuls on a memset scratch tile right after
    the framework preamble so the HAM clock-gate opens before real work.
"""

import numpy as np
from contextlib import ExitStack

import ml_dtypes
import concourse.bass as bass
import concourse.mybir as mybir
import concourse.tile as tile
from concourse import bacc, bass_utils

F32 = mybir.dt.float32
BF16 = mybir.dt.bfloat16
BF16_NP = ml_dtypes.bfloat16

ENS, N_UNITS, N_HID, IN_DIM, OUT_DIM, BATCH = 10, 256, 7, 6, 201, 16384
N_CORES = 8
BC = BATCH // N_CORES          # 2048 batch rows per core
N_WARM = 8                     # pre-warm matmuls for the HAM clock gate

_CACHE = {}


def build_program():
    nc = bacc.Bacc("TRN2", debug=False)

    # x^T (+ones row) replicated at partition offsets 0/32/64/96 so layer-0's
    # K=7 matmuls can run 4-concurrent via PE row-group tiling.
    xq = nc.dram_tensor("xq", (128, BC), BF16, kind="ExternalInput").ap()
    w0 = nc.dram_tensor("w0", (ENS, 128, 128), BF16, kind="ExternalInput").ap()
    wh = nc.dram_tensor("wh", (ENS, 128, N_HID * 2 * N_UNITS), BF16, kind="ExternalInput").ap()
    bh = nc.dram_tensor("bh", (ENS, 128, N_HID * 2), F32, kind="ExternalInput").ap()
    vw = nc.dram_tensor("vw", (ENS, 128, 2 * 256), BF16, kind="ExternalInput").ap()
    bp = nc.dram_tensor("bp", (128, 2), F32, kind="ExternalInput").ap()
    yt = nc.dram_tensor("yt", (256, BC), F32, kind="ExternalOutput").ap()

    add = mybir.AluOpType.add
    mx = mybir.AluOpType.max
    relu = mybir.ActivationFunctionType.Relu
    ident = mybir.ActivationFunctionType.Identity

    with ExitStack() as ctx:
        tc = ctx.enter_context(tile.TileContext(nc))
        const = ctx.enter_context(tc.tile_pool(name="const", bufs=1))
        wpool = ctx.enter_context(tc.tile_pool(name="w", bufs=2))
        vpool = ctx.enter_context(tc.tile_pool(name="v", bufs=1))
        hpool = ctx.enter_context(tc.tile_pool(name="h", bufs=2))
        hfpool = ctx.enter_context(tc.tile_pool(name="hf", bufs=1))
        spool = ctx.enter_context(tc.tile_pool(name="stage", bufs=1))
        pspool = ctx.enter_context(tc.tile_pool(name="ps", bufs=8, space="PSUM"))

        x_t = const.tile([128, BC], BF16)
        bp_t = const.tile([128, 2], F32)
        scratch = const.tile([128, 512], BF16)
        v_all = vpool.tile([128, ENS * 512], BF16)
        hf = {}

        # ---- PE pre-warm: dummy matmuls on zeroed scratch, result unread ----
        nc.gpsimd.memset(scratch, 0.0)
        ps_warm = pspool.tile([128, 512], F32, tag="ps")
        for _ in range(N_WARM):
            nc.tensor.matmul(ps_warm, lhsT=scratch[:, 0:128],
                             rhs=scratch, start=True, stop=True)

        def relu_tile(engine_is_act, dst, ps, bias_ap):
            if engine_is_act:
                nc.scalar.activation(out=dst, in_=ps, func=relu,
                                     bias=bias_ap if bias_ap is not None else 0.0)
            elif bias_ap is not None:
                nc.vector.tensor_scalar(out=dst, in0=ps, scalar1=bias_ap,
                                        scalar2=0.0, op0=add, op1=mx)
            else:
                nc.vector.tensor_scalar(out=dst, in0=ps, scalar1=0.0,
                                        scalar2=None, op0=mx)

        for e in range(ENS):
            w0_t = wpool.tile([128, 128], BF16, tag="w0")
            nc.sync.dma_start(out=w0_t, in_=w0[e])
            if e == 0:
                # issue x in b-slices so layer-0's first matmuls wait only on
                # their own slice.
                for bt in range(4):
                    nc.sync.dma_start(out=x_t[:, bt * 512:(bt + 1) * 512],
                                      in_=xq[:, bt * 512:(bt + 1) * 512])
                nc.sync.dma_start(out=bp_t, in_=bp)
            wh_t = wpool.tile([128, N_HID * 2 * N_UNITS], BF16, tag="wh")
            for l in range(N_HID):
                nc.gpsimd.dma_start(out=wh_t[:, l * 512:(l + 1) * 512],
                                    in_=wh[e][:, l * 512:(l + 1) * 512])
            bh_t = wpool.tile([128, N_HID * 2], F32, tag="bh")
            nc.sync.dma_start(out=bh_t, in_=bh[e])
            nc.gpsimd.dma_start(out=v_all[:, e * 512:(e + 1) * 512], in_=vw[e])

            # ---- layer 0: x^T (7, BC) -> h (2x128, BC); bias folded in.
            # 4 K=7 matmuls run concurrently in the 4 PE row groups
            # (row-group i holds weights for oc=i//2, streams bt parity i%2).
            # Relus are per-512-slice, issued right after each pack/pair and
            # alternated ACT/DVE to minimize latency (it gates PSUM reuse and
            # the next layer's first matmuls).
            h_cur = [hpool.tile([128, BC], BF16, tag=f"h{kc}", name=f"h{kc}_e{e}")
                     for kc in range(2)]
            eng = 0
            for j in range(2):
                pst = {}
                for i in range(4):
                    oc, p = i // 2, i % 2
                    bt = 2 * j + p
                    pst[i] = pspool.tile([128, 512], F32, tag="ps",
                                         name=f"ps{i}_{j}_e{e}L0")
                    nc.tensor.matmul(
                        pst[i],
                        lhsT=w0_t[32 * i:32 * i + IN_DIM + 1, :],
                        rhs=x_t[32 * i:32 * i + IN_DIM + 1,
                                bt * 512:(bt + 1) * 512],
                        start=True, stop=True, tile_position=(32 * i, 0))
                for i in range(4):
                    oc, p = i // 2, i % 2
                    bt = 2 * j + p
                    relu_tile(eng % 2 == 0,
                              h_cur[oc][:, bt * 512:(bt + 1) * 512],
                              pst[i], None)
                    eng += 1

            # ---- 7 hidden layers: K=256 (2 chunks), M=256 (2 chunks) ----
            for l in range(N_HID):
                if l < N_HID - 1:
                    h_nxt = [hpool.tile([128, BC], BF16, tag=f"h{kc}",
                                        name=f"h{kc}_e{e}l{l}")
                             for kc in range(2)]
                else:
                    h_nxt = [hfpool.tile([128, BC], BF16, tag=f"hf{e}_{kc}",
                                         name=f"hf{e}_{kc}")
                             for kc in range(2)]
                    for kc in range(2):
                        hf[(e, kc)] = h_nxt[kc]
                base = l * 2 * N_UNITS
                eng = 0
                for bt in range(4):
                    hsl = slice(bt * 512, (bt + 1) * 512)
                    for oc in range(2):
                        ps = pspool.tile([128, 512], F32, tag="ps",
                                         name=f"ps{oc}_{bt}_e{e}l{l}")
                        c0 = base + oc * 128
                        c1 = base + N_UNITS + oc * 128
                        nc.tensor.matmul(ps, lhsT=wh_t[:, c0:c0 + 128],
                                         rhs=h_cur[0][:, hsl],
                                         start=True, stop=False)
                        nc.tensor.matmul(ps, lhsT=wh_t[:, c1:c1 + 128],
                                         rhs=h_cur[1][:, hsl],
                                         start=False, stop=True)
                        relu_tile(eng % 2 == 0, h_nxt[oc][:, hsl], ps,
                                  bh_t[:, l * 2 + oc:l * 2 + oc + 1])
                        eng += 1
                h_cur = h_nxt

        # ---- final layer: out[o', b] = sum_e sum_kc V[e][kc].T @ hf[e][kc] ----
        # Ensemble sum accumulates in PSUM (20 matmuls per bank).
        for g, (bt, oc) in enumerate([(bt, oc) for bt in range(4)
                                      for oc in range(2)]):
            ps = pspool.tile([128, 512], F32, tag="ps", name=f"psf{g}")
            hsl = slice(bt * 512, (bt + 1) * 512)
            for e in range(ENS):
                for kc in range(2):
                    c = e * 512 + kc * 256 + oc * 128
                    nc.tensor.matmul(ps, lhsT=v_all[:, c:c + 128],
                                     rhs=hf[(e, kc)][:, hsl],
                                     start=(e == 0 and kc == 0),
                                     stop=(e == ENS - 1 and kc == 1))
            stage = spool.tile([128, 512], F32, tag=f"s{g}")
            if g % 2 == 0:
                nc.scalar.activation(out=stage, in_=ps, func=ident,
                                     bias=bp_t[:, oc:oc + 1])
            else:
                nc.vector.tensor_scalar(out=stage, in0=ps,
                                        scalar1=bp_t[:, oc:oc + 1],
                                        scalar2=None, op0=add)
            dma_eng = [nc.sync, nc.gpsimd][g % 2]
            dma_eng.dma_start(out=yt[oc * 128:(oc + 1) * 128, hsl], in_=stage)

    nc.compile()
    return nc


def prepare_inputs(x, W0, b0, Wh, bh, Wf, bf):
    """Host-side weight refactoring + per-core sharding. Exact fp32 linear
    algebra for the folds; bf16 quantization only at the very end."""
    x = np.asarray(x, np.float32)
    W0 = np.asarray(W0, np.float32)
    b0 = np.asarray(b0, np.float32)
    Wh = np.asarray(Wh, np.float32)
    bh = np.asarray(bh, np.float32)
    Wf = np.asarray(Wf, np.float32)
    bf = np.asarray(bf, np.float32)

    # layer 0 with bias folded: lhsT rows = 6 inputs + ones row; packed into
    # the 4 PE row groups (groups 0,1 -> oc0 weights; groups 2,3 -> oc1).
    w0a = np.concatenate([W0, b0[:, None, :]], axis=1)  # (ENS, 7, 256)
    w0q = np.zeros((ENS, 128, 128), np.float32)
    for i in range(4):
        w0q[:, 32 * i:32 * i + IN_DIM + 1, :] = \
            w0a[:, :, (i // 2) * 128:(i // 2) * 128 + 128]
    w0q = np.ascontiguousarray(w0q).astype(BF16_NP)

    # hidden weights -> [e, p, (l, kc, o)]
    whh = (Wh.transpose(1, 0, 2, 3)              # (e, l, h, o)
             .reshape(ENS, N_HID, 2, 128, N_UNITS)
             .transpose(0, 3, 1, 2, 4)           # (e, p, l, kc, o)
             .reshape(ENS, 128, N_HID * 2 * N_UNITS))
    whh = np.ascontiguousarray(whh).astype(BF16_NP)

    # hidden biases -> [e, p, (l, oc)]
    bhh = (bh.transpose(1, 0, 2)                 # (e, l, o)
             .reshape(ENS, N_HID, 2, 128)
             .transpose(0, 3, 1, 2)              # (e, p, l, oc)
             .reshape(ENS, 128, N_HID * 2))
    bhh = np.ascontiguousarray(bhh)

    # fold the strided channel-mix mean into the final weights:
    # out[b, o'] = 0.1 * sum_j yflat[b, o'*10+j],  yflat col c = e*201+o
    C = ENS * OUT_DIM
    M = np.zeros((C, OUT_DIM), np.float32)
    M[np.arange(C), np.arange(C) // ENS] = 1.0 / ENS
    Me = M.reshape(ENS, OUT_DIM, OUT_DIM)
    V = np.einsum('eho,eoc->ehc', Wf, Me)        # (ENS, 256, 201)
    bpv = bf.reshape(C) @ M                      # (201,)

    Vp = np.zeros((ENS, N_UNITS, 256), np.float32)
    Vp[:, :, :OUT_DIM] = V
    vww = (Vp.reshape(ENS, 2, 128, 256)
             .transpose(0, 2, 1, 3)              # (e, p, kc, o')
             .reshape(ENS, 128, 2 * 256))
    vww = np.ascontiguousarray(vww).astype(BF16_NP)

    bp_pad = np.zeros(256, np.float32)
    bp_pad[:OUT_DIM] = bpv
    bp_t = np.ascontiguousarray(bp_pad.reshape(2, 128).T)  # (128, 2)

    ones = np.ones((1, BC), np.float32)
    in_maps = []
    for c in range(N_CORES):
        xs = x[c * BC:(c + 1) * BC]              # (BC, 6)
        xt = np.concatenate([xs.T, ones], axis=0)  # (7, BC)
        xqc = np.zeros((128, BC), np.float32)
        for i in range(4):
            xqc[32 * i:32 * i + IN_DIM + 1, :] = xt
        xqc = np.ascontiguousarray(xqc).astype(BF16_NP)
        in_maps.append({
            "xq": xqc, "w0": w0q, "wh": whh, "bh": bhh, "vw": vww, "bp": bp_t,
        })
    return in_maps


def run(in_maps, trace=False, tmpdir=None):
    if "nc" not in _CACHE:
        _CACHE["nc"] = build_program()
    nc = _CACHE["nc"]
    res = bass_utils.run_bass_kernel_spmd(
        nc, in_maps, core_ids=list(range(N_CORES)), trace=trace, tmpdir=tmpdir)
    return res


def kernel(x, W0, b0, Wh, bh, Wf, bf):
    in_maps = prepare_inputs(x, W0, b0, Wh, bh, Wf, bf)
    res = run(in_maps)
    out = np.empty((BATCH, OUT_DIM), np.float32)
    for c in range(N_CORES):
        out[c * BC:(c + 1) * BC, :] = res.results[c]["yt"][:OUT_DIM].T
    return out
